# revision 12
# baseline (speedup 1.0000x reference)
"""Trainium2 Bass kernel for nn_Projection: out = [(1-s)*x, s],
s = -(1-||x||^2)/(1+||x||^2) per row.

Identity used: with sq = sum(x^2), s = (sq-1)/(sq+1) = 1 - 2/(1+sq).
Let t = 2/(1+sq). Then out = [t*x, 1-t].

HBM-bandwidth bound (elementwise over 512MB in / 516MB out). Gate is
rel_err < 2e-2, so all HBM traffic is bf16 (measured max rel err
~1.2e-2): the host rounds x to bf16, the device computes and stores
bf16, the host upcasts to f32. Halves HBM traffic vs f32.

Layout trick: tiles are d-major in SBUF ([P, D, blk], host pre/post
transposes the per-tile element order). This makes every hot DVE op
eligible for the 2-byte packed 2x perf mode (innermost step 1 on all
operands, including the per-row t broadcast which is stride-0 only
in the middle dim):
  - row-sum of x^2: 7-level fp16 fold tree over d, each fold a 2x TT
  - t*x multiply: one 2x TT with t16 broadcast over d
GpSimd stays idle: any GpSimd op would serialize with DVE 2x ops on
the exclusive shared SBUF port pair.

Software-pipelined with a 2-iteration skew so the in-order ACT/DVE
queues never stall on each other:
  ACT iter k: u_{k-2}=sq+0.5, s_col_{k-3}, square_k   (+ load k+PRE)
  DVE iter k: folds_{k-1}, recip_{k-2}, cast_{k-2}, mul_{k-2}
  SP  iter k: store_{k-2}

Sharding: pure data parallel over rows across 8 NeuronCores.
Per-core row map: row = p*K + i*blk + j  (p partition, i iteration).
"""

import sys

for _p in ("/opt/trn_rl_repo", "/opt/trn_rl_repo/concourse"):
    if _p not in sys.path:
        sys.path.insert(0, _p)

import ml_dtypes
import numpy as np

import concourse.bacc as bacc
import concourse.tile as tile
from concourse import mybir
from concourse.bass_utils import run_bass_kernel_spmd

N, D = 1048576, 128
N_CORES = 8
R = N // N_CORES   # 131072 rows per core
P = 128            # SBUF partitions
K = R // P         # 1024 rows per partition
BLK = 64
NITER = K // BLK
BF16 = mybir.dt.bfloat16
FP16 = mybir.dt.float16
F32 = mybir.dt.float32
NP_BF16 = np.dtype(ml_dtypes.bfloat16)


def build_nc(blk: int = BLK, pre: int = 2):
    k_rows = K
    niter = k_rows // blk
    fd = blk * D

    nc = bacc.Bacc(trn_type="TRN2")
    x = nc.dram_tensor("x", [niter, P, fd], BF16, kind="ExternalInput")
    tx = nc.dram_tensor("tx", [niter, P, fd], BF16, kind="ExternalOutput")
    s = nc.dram_tensor("s", [P, k_rows], BF16, kind="ExternalOutput")
    xv = x.ap()
    tv = tx.ap()

    with tile.TileContext(nc) as tc:
        with (
            tc.tile_pool(name="xin", bufs=pre + 4) as x_pool,
            tc.tile_pool(name="out", bufs=3) as o_pool,
            tc.tile_pool(name="sqp", bufs=2) as sq_pool,
            tc.tile_pool(name="small", bufs=4) as small_pool,
            tc.tile_pool(name="singles", bufs=1) as singles,
        ):
            s_all = singles.tile([P, k_rows], BF16)

            x_t = {}     # j -> x tile
            xsq = {}     # j -> squared tile
            sqs = {}     # j -> row-sum (fp16 [P, blk])
            us = {}      # j -> u = sq + 0.5 (f32)
            t32s = {}    # j -> 1/u (f32)

            def load(j):
                # Alternate rings by parity: during ramp (loads only) and
                # drain (stores only) the single active direction can then
                # use BOTH HWDGE rings (~one ring alone caps at ~240GB/s).
                x_t[j] = x_pool.tile([P, D, blk], BF16, tag="x", name="x_t")
                eng = nc.scalar if j % 2 == 0 else nc.sync
                eng.dma_start(out=x_t[j], in_=xv[j])

            def square(j):
                xsq[j] = sq_pool.tile([P, D, blk], FP16, tag="xsq", name="xsq")
                nc.scalar.activation(
                    out=xsq[j], in_=x_t[j],
                    func=mybir.ActivationFunctionType.Square,
                    scale=0.7071067811865476,
                )

            def folds(j):
                # In-place fold tree: each level writes its sum into the
                # lower half of the region it read (the write pointer
                # trails both read pointers, so this is race-free).
                a = xsq[j]
                d = D
                while d > 2:
                    d //= 2
                    nc.vector.tensor_add(
                        a[:, 0:d, :], a[:, 0:d, :], a[:, d:2 * d, :])
                sqs[j] = small_pool.tile([P, blk], FP16, tag="sq", name="sq")
                nc.vector.tensor_add(
                    sqs[j][:, :].unsqueeze(1), a[:, 0:1, :], a[:, 1:2, :])
                del xsq[j]

            def u_of(j):
                us[j] = small_pool.tile([P, blk], F32, tag="u", name="u")
                nc.scalar.activation(
                    out=us[j], in_=sqs[j],
                    func=mybir.ActivationFunctionType.Copy, bias=0.5,
                )
                del sqs[j]

            def tail(j):
                t32s[j] = small_pool.tile([P, blk], F32, tag="t32", name="t32")
                nc.vector.reciprocal_approx_fast(out=t32s[j], in_=us[j])
                del us[j]
                t16 = small_pool.tile([P, blk], BF16, tag="t16")
                nc.vector.tensor_copy(t16, t32s[j])
                out_t = o_pool.tile([P, D, blk], BF16, tag="out")
                nc.vector.tensor_mul(
                    out_t, x_t[j],
                    t16[:, :].unsqueeze(1).broadcast_to([P, D, blk]))
                del x_t[j]
                eng = nc.sync if j % 2 == 0 else nc.scalar
                eng.dma_start(out=tv[j], in_=out_t)

            def s_col(j):
                nc.scalar.activation(
                    out=s_all[:, j * blk:(j + 1) * blk], in_=t32s[j],
                    func=mybir.ActivationFunctionType.Copy,
                    bias=1.0, scale=-1.0,
                )
                del t32s[j]

            for j in range(pre):
                load(j)

            for k in range(niter + 3):
                # ACT: smalls first so they never queue behind the square
                if 0 <= k - 2 < niter:
                    u_of(k - 2)
                if 0 <= k - 3 < niter:
                    s_col(k - 3)
                if k < niter:
                    if k + pre < niter:
                        load(k + pre)
                    square(k)
                # DVE: tail (mul+store) first so the store DMA issues early
                # in the iteration; folds of the next tile follow.
                if 0 <= k - 2 < niter:
                    tail(k - 2)
                if 0 <= k - 1 < niter:
                    folds(k - 1)

            nc.sync.dma_start(out=s.ap(), in_=s_all)

    nc.compile()
    return nc


def host_pack(x16_flat: np.ndarray) -> np.ndarray:
    """[N_CORES*R, D] bf16 row-major -> [N_CORES, NITER, P, D*BLK] d-major."""
    v = x16_flat.view(np.uint16)
    v = v.reshape(N_CORES, P, NITER, BLK, D)
    v = np.ascontiguousarray(v.transpose(0, 2, 1, 4, 3))
    return v.reshape(N_CORES, NITER, P, D * BLK).view(NP_BF16)


def host_unpack_tx(txd: np.ndarray) -> np.ndarray:
    """[NITER, P, D*BLK] d-major -> [R, D] f32."""
    v = txd.view(np.uint16).reshape(NITER, P, D, BLK)
    v = np.ascontiguousarray(v.transpose(1, 0, 3, 2))
    return v.reshape(R, D).view(NP_BF16).astype(np.float32)


_nc_cache: dict = {}


def _get_nc():
    if "nc" not in _nc_cache:
        _nc_cache["nc"] = build_nc()
    return _nc_cache["nc"]


def kernel(x) -> np.ndarray:
    x = np.asarray(x)
    assert x.shape == (N, D), x.shape
    x16 = np.ascontiguousarray(x.astype(NP_BF16))
    packed = host_pack(x16)
    nc = _get_nc()
    in_maps = [{"x": packed[c]} for c in range(N_CORES)]
    res = run_bass_kernel_spmd(nc, in_maps, core_ids=list(range(N_CORES)))
    out = np.empty((N, D + 1), dtype=np.float32)
    for c, r in enumerate(res.results):
        out[c * R:(c + 1) * R, :D] = host_unpack_tx(r["tx"])
        out[c * R:(c + 1) * R, D] = r["s"].reshape(R).astype(np.float32)
    return out
